# revision 5
# baseline (speedup 1.0000x reference)
"""GCN (3-layer, N=100k nodes, E=1.6M edges) on 8 Trainium2 NeuronCores.

Strategy (self-contained, shapes hardcoded for this problem):
  - Reformulate each layer with associativity:
        h_{l+1} = relu(Dinv (A+I) Dinv h_l W + b)
                = relu(Dinv * [ SpMM(u_l) @ W ] + b),   u_l = Dinv h_l
    and keep u_l = relu(dinv^2 * z + dinv*b) directly (dinv >= 0 commutes
    with relu), so the only per-node scalings are per-partition.
  - Shard the 784 dest-row tiles (128 rows each) across 8 cores, balanced by
    edge count (sorted + round-robin), identical SPMD program on all cores.
  - Each layer: AllGather u shards -> per-core bf16 table [100352, 128] in
    local HBM; gather edge source rows with dma_gather (int16 indices into
    four 25088-row quarters); one-hot selection matrices (iota + is_equal)
    route gathered rows to dest rows via PE matmuls accumulating in PSUM in
    transposed orientation [feat, dest]; then a plain matmul applies W.
"""
import sys
sys.path.insert(0, "/opt/trn_rl_repo")
import numpy as np
import ml_dtypes

N = 100000
F = 128
FO = 64
P = 128
NCORES = 8
SLOTS = 98                 # dest tiles per core
NPAD = NCORES * SLOTS * P  # 100352
NT = NPAD // P             # 784 tiles
QROWS = NPAD // 4          # 25088 rows per gather quarter (< int16 max)
GRP = 6                    # dest tiles processed per PSUM group
import os as _os
KBMAX = int(_os.environ.get("GCN_KBMAX", "32"))   # max 128-edge blocks per dma_gather call
SINGLE_PACKET = _os.environ.get("GCN_SP", "1") == "1"

LAST_EXEC_NS = None


def _prep(x, row, col):
    """Host-side sharding/scheduling. Returns per-core arrays + shared schedule."""
    deg = np.bincount(row, minlength=N).astype(np.float64) + 1.0
    dinv = (1.0 / np.sqrt(deg)).astype(np.float32)

    gt = row >> 7                                   # dest tile of each edge
    cnt = np.bincount(gt, minlength=NT)
    order = np.argsort(-cnt, kind="stable")         # tiles by edge count desc
    core_of = np.empty(NT, np.int64)
    slot_of = np.empty(NT, np.int64)
    core_of[order] = np.tile(np.arange(NCORES), SLOTS)
    slot_of[order] = np.repeat(np.arange(SLOTS), NCORES)

    nodes = np.arange(NPAD)
    g_n = nodes >> 7
    pos = core_of[g_n] * (SLOTS * P) + slot_of[g_n] * P + (nodes & 127)

    dinv_full = np.ones(NPAD, np.float32)
    dinv_full[:N] = dinv

    colpos = pos[col]
    q_e = colpos // QROWS
    qloc_e = (colpos % QROWS).astype(np.int16)
    c_e = core_of[gt]
    s_e = slot_of[gt]
    dloc_e = (row & 127).astype(np.uint8)

    key = (c_e * SLOTS + s_e) * 4 + q_e             # [E]
    sort_idx = np.argsort(key, kind="stable")
    key_s = key[sort_idx]
    cnts = np.bincount(key, minlength=NCORES * SLOTS * 4).reshape(NCORES, SLOTS, 4)
    B = -(-cnts // P)                               # blocks needed [core, slot, q]
    Bq = B.max(axis=0)                              # shared schedule [slot, q]

    # global block emission order: groups of GRP slots, quarter-major inside
    groups = [list(range(g, min(g + GRP, SLOTS))) for g in range(0, SLOTS, GRP)]
    blkoff = np.zeros((SLOTS, 4), np.int64)
    emit = []                                       # [(slot, q)] in block order
    off = 0
    sched = []   # per group: dict(slots=[...], qcalls={q: [(b0, nb, [(slot,last)..])]})
    slot_nblocks = Bq.sum(axis=1)
    for grp in groups:
        qcalls = {}
        for q in range(4):
            blocks = []                             # (slot, islast) per block
            for s in grp:
                nb_s = Bq[s, q]
                if nb_s == 0:
                    continue
                blkoff[s, q] = off + len(blocks)
                done_before = Bq[s, :q].sum()
                for j in range(nb_s):
                    islast = (done_before + j) == slot_nblocks[s] - 1
                    blocks.append((s, islast))
            calls = []
            b0 = 0
            while b0 < len(blocks):
                nb = min(KBMAX, len(blocks) - b0)
                calls.append((off + b0, nb, blocks[b0 : b0 + nb]))
                b0 += nb
            qcalls[q] = calls
            off += len(blocks)
        sched.append({"slots": grp, "qcalls": qcalls})
    TB = int(off)

    # per-core lane arrays
    qidx_lanes = np.zeros((NCORES, TB * P), np.int16)
    dloc_lanes = np.full((NCORES, TB * P), 255, np.int16)
    starts = np.zeros(NCORES * SLOTS * 4 + 1, np.int64)
    np.cumsum(np.bincount(key, minlength=NCORES * SLOTS * 4), out=starts[1:])
    rank_in_seg = np.arange(len(key_s)) - starts[key_s]
    c_s = key_s // (SLOTS * 4)
    s_s = (key_s // 4) % SLOTS
    q_s = key_s % 4
    blk = blkoff[s_s, q_s] + rank_in_seg // P
    lane = rank_in_seg % P
    dst = blk * P + lane
    e_ids = sort_idx
    qidx_lanes[c_s, dst] = qloc_e[e_ids]
    dloc_lanes[c_s, dst] = dloc_e[e_ids]

    # pack indices per call: [128, 8*TB] with 16-partition wrap per call
    def pack_core(qi):
        colsets = []
        for grp_s in sched:
            for q in range(4):
                for (b0, nb, _blocks) in grp_s["qcalls"][q]:
                    seg = qi[b0 * P : (b0 + nb) * P]
                    arr = seg.reshape(-1, 16).T          # [16, 8*nb]
                    colsets.append(np.tile(arr, (8, 1)))
        return np.concatenate(colsets, axis=1).astype(np.int16)

    qidx_packed = np.stack([pack_core(qidx_lanes[c]) for c in range(NCORES)])
    dloc_in = (
        dloc_lanes.reshape(NCORES, TB, P)
        .transpose(0, 2, 1)
        .astype(np.float32)
        .astype(ml_dtypes.bfloat16)
    )                                                # [NCORES, 128, TB]

    # per-core u0 shard, dinv columns
    x_pad = np.zeros((NPAD, F), np.float32)
    x_pad[:N] = x
    u0_full = (dinv_full[:, None] * x_pad).astype(ml_dtypes.bfloat16)
    u0_shards = np.empty((NCORES, SLOTS * P, F), ml_dtypes.bfloat16)
    u0_shards.reshape(NCORES, SLOTS * P, F)[core_of[g_n], slot_of[g_n] * P + (nodes & 127)] = u0_full

    dv = np.empty((NCORES, SLOTS * P), np.float32)
    dv[core_of[g_n], slot_of[g_n] * P + (nodes & 127)] = dinv_full
    dinv_cols = dv.reshape(NCORES, SLOTS, P).transpose(0, 2, 1).copy()   # [c,128,SLOTS]
    dinv2_cols = (dinv_cols ** 2).copy()
    rdinv_rows = (1.0 / dv.reshape(NCORES, SLOTS, P)).astype(np.float32) # [c,SLOTS,128]

    return dict(
        sched=sched, TB=TB, Bq=Bq,
        qidx=qidx_packed, dloc=dloc_in, u0=u0_shards,
        dinv_cols=dinv_cols, dinv2_cols=dinv2_cols, rdinv_rows=rdinv_rows,
        core_of=core_of, slot_of=slot_of,
    )


def _build_program(prep, trace=False):
    import os
    from concourse import bass, bacc, mybir, tile
    from concourse.masks import make_identity
    NLAYERS = int(os.environ.get("GCN_LAYERS", "3"))
    NOGATHER = os.environ.get("GCN_NOGATHER", "0") == "1"
    NOCC = os.environ.get("GCN_NOCC", "0") == "1"

    bf16 = mybir.dt.bfloat16
    f32 = mybir.dt.float32

    sched = prep["sched"]
    TB = prep["TB"]

    nc = bacc.Bacc("TRN2", target_bir_lowering=False, debug=False, num_devices=NCORES)

    u0_in = nc.dram_tensor("u0", [SLOTS * P, F], bf16, kind="ExternalInput")
    qidx_in = nc.dram_tensor("qidx", [P, 8 * TB], mybir.dt.int16, kind="ExternalInput")
    dloc_in = nc.dram_tensor("dloc", [P, TB], bf16, kind="ExternalInput")
    w_ins = [
        nc.dram_tensor("W0", [F, F], bf16, kind="ExternalInput"),
        nc.dram_tensor("W1", [F, F], bf16, kind="ExternalInput"),
        nc.dram_tensor("W2", [F, FO], bf16, kind="ExternalInput"),
    ]
    b_ins = [
        nc.dram_tensor("b0", [1, F], bf16, kind="ExternalInput"),
        nc.dram_tensor("b1", [1, F], bf16, kind="ExternalInput"),
        nc.dram_tensor("b2", [1, FO], bf16, kind="ExternalInput"),
    ]
    dinvc_in = nc.dram_tensor("dinvc", [P, SLOTS], f32, kind="ExternalInput")
    dinv2c_in = nc.dram_tensor("dinv2c", [P, SLOTS], f32, kind="ExternalInput")
    rdinv_in = nc.dram_tensor("rdinv", [1, SLOTS * P], bf16, kind="ExternalInput")
    out_dram = nc.dram_tensor("out", [SLOTS * P, FO], f32, kind="ExternalOutput")

    with tile.TileContext(nc) as tc:
        with tc.tile_pool(name="dram", bufs=1, space="DRAM") as dram, \
             tc.tile_pool(name="consts", bufs=1) as consts, \
             tc.tile_pool(name="uarena", bufs=1) as uarena, \
             tc.tile_pool(name="gpool", bufs=int(_os.environ.get("GCN_GBUFS","4"))) as gpool, \
             tc.tile_pool(name="spool", bufs=int(_os.environ.get("GCN_SBUFS","4"))) as spool, \
             tc.tile_pool(name="uown0", bufs=3) as uown0, \
             tc.tile_pool(name="ytpool", bufs=3) as ytpool, \
             tc.tile_pool(name="opool", bufs=3) as opool, \
             tc.tile_pool(name="spmmps", bufs=GRP, space="PSUM") as spmmps, \
             tc.tile_pool(name="zpsp", bufs=2, space="PSUM") as zpsp:

            # ---- constants ----
            ident = consts.tile([P, P], bf16)
            make_identity(nc, ident[:])
            iota_i = consts.tile([P, P], mybir.dt.int32)
            nc.gpsimd.iota(iota_i[:], pattern=[[1, P]], base=0, channel_multiplier=0)
            iota_b = consts.tile([P, P], bf16)
            nc.vector.tensor_copy(out=iota_b[:], in_=iota_i[:])
            qidx_sb = consts.tile([P, 8 * TB], mybir.dt.int16)
            nc.sync.dma_start(out=qidx_sb[:], in_=qidx_in.ap()[:])
            dloc_sb = consts.tile([P, TB], bf16)
            nc.sync.dma_start(out=dloc_sb[:], in_=dloc_in.ap()[:])
            w_sb = []
            for li, w in enumerate(w_ins):
                wt = consts.tile(list(w.shape), bf16, name=f"w{li}sb")
                nc.sync.dma_start(out=wt[:], in_=w.ap()[:])
                w_sb.append(wt)
            b_sb = []
            for li, b in enumerate(b_ins):
                bt = consts.tile(list(b.shape), bf16, name=f"b{li}sb")
                nc.sync.dma_start(out=bt[:], in_=b.ap()[:])
                b_sb.append(bt)
            dinvc_sb = consts.tile([P, SLOTS], f32)
            nc.sync.dma_start(out=dinvc_sb[:], in_=dinvc_in.ap()[:])
            dinv2c_sb = consts.tile([P, SLOTS], f32)
            nc.sync.dma_start(out=dinv2c_sb[:], in_=dinv2c_in.ap()[:])
            rdinv_sb = consts.tile([1, SLOTS * P], bf16)
            nc.sync.dma_start(out=rdinv_sb[:], in_=rdinv_in.ap()[:])

            # ---- u arenas (layer outputs kept resident in SBUF) ----
            uA = [uarena.tile([P, F], bf16, tag=f"uA{s}", name=f"uA{s}") for s in range(SLOTS)]
            uB = [uarena.tile([P, F], bf16, tag=f"uB{s}", name=f"uB{s}") for s in range(SLOTS)]

            # ---- AG staging + tables ----
            stages = [dram.tile([SLOTS * P, F], bf16, name=f"stage{l}") for l in range(3)]
            tables = [dram.tile([NPAD, F], bf16, addr_space="Shared", name=f"table{l}")
                      for l in range(3)]

            # layer 0 staging comes straight from the input shard
            nc.sync.dma_start(out=stages[0][:], in_=u0_in.ap()[:])

            rg = [list(range(NCORES))]
            for layer in range(NLAYERS):
                last = layer == NLAYERS - 1
                fo = FO if last else F
                u_in_tiles = None if layer == 0 else (uA if layer == 1 else uB)
                u_out_tiles = uA if layer == 0 else (uB if layer == 1 else None)

                if not NOCC:
                    nc.gpsimd.collective_compute(
                        "AllGather", mybir.AluOpType.bypass,
                        replica_groups=rg,
                        ins=[stages[layer].opt()], outs=[tables[layer].opt()],
                    )
                else:
                    nc.sync.dma_start(
                        out=tables[layer].opt()[:SLOTS * P, :],
                        in_=stages[layer].opt()[:])

                for grp_s in sched:
                    psums = {}
                    uowns = {}
                    for s in grp_s["slots"]:
                        if layer == 0:
                            ut = uown0.tile([P, F], bf16, tag="u0own", name=f"u0own_{layer}_{s}")
                            nc.sync.dma_start(
                                out=ut[:], in_=u0_in.ap()[s * P : (s + 1) * P, :])
                            uowns[s] = ut
                        else:
                            uowns[s] = u_in_tiles[s]
                        ps = spmmps.tile([P, P], f32, tag="spmm", name=f"ps_{layer}_{s}",
                                         space="PSUM")
                        psums[s] = ps
                        # self-loop: psum[fi, dest] += u_own.T
                        nc.tensor.matmul(
                            out=ps[:], lhsT=uowns[s][:], rhs=ident[:],
                            start=True,
                            stop=(NOGATHER or int(prep["Bq"][s].sum()) == 0),
                        )
                    for q in ([] if NOGATHER else range(4)):
                        for (b0, nb, blocks) in grp_s["qcalls"][q]:
                            gt_ = gpool.tile([P, nb, F], bf16, tag="g", name=f"g_{layer}_{b0}")
                            nc.gpsimd.dma_gather(
                                out_ap=gt_[:],
                                in_ap=tables[layer].opt()[q * QROWS : (q + 1) * QROWS, :],
                                idxs_ap=qidx_sb[:, 8 * b0 : 8 * (b0 + nb)],
                                num_idxs=nb * P,
                                num_idxs_reg=nb * P,
                                elem_size=F,
                                single_packet=SINGLE_PACKET,
                            )
                            st_ = spool.tile([P, nb, P], bf16, tag="s", name=f"s_{layer}_{b0}")
                            nc.vector.tensor_tensor(
                                out=st_[:],
                                in0=dloc_sb[:, b0 : b0 + nb]
                                    .rearrange("p (a b) -> p a b", b=1)
                                    .to_broadcast([P, nb, P]),
                                in1=iota_b[:]
                                    .rearrange("p (a b) -> p a b", a=1)
                                    .to_broadcast([P, nb, P]),
                                op=mybir.AluOpType.is_equal,
                            )
                            for j, (s, islast) in enumerate(blocks):
                                nc.tensor.matmul(
                                    out=psums[s][:],
                                    lhsT=gt_[:, j, :], rhs=st_[:, j, :],
                                    start=False, stop=islast,
                                )
                    for s in grp_s["slots"]:
                        yt = ytpool.tile([P, P], bf16, tag="yt", name=f"yt_{layer}_{s}")
                        nc.vector.tensor_copy(out=yt[:], in_=psums[s][:])
                        zps = zpsp.tile([P, fo], f32, tag="z", name=f"z_{layer}_{s}",
                                        space="PSUM")
                        nc.tensor.matmul(out=zps[:],
                                         lhsT=yt[:], rhs=w_sb[2 if last else layer][:],
                                         start=True, stop=False)
                        nc.tensor.matmul(out=zps[:], lhsT=rdinv_sb[0:1, s * P : (s + 1) * P],
                                         rhs=b_sb[2 if last else layer][:], start=False, stop=True)
                        if not last:
                            un = u_out_tiles[s]
                            nc.scalar.activation(
                                out=un[:], in_=zps[:],
                                func=mybir.ActivationFunctionType.Relu,
                                scale=dinv2c_sb[:, s : s + 1],
                            )
                            nc.sync.dma_start(
                                out=stages[layer + 1][s * P : (s + 1) * P, :],
                                in_=un[:])
                        else:
                            ot = opool.tile([P, FO], f32, tag="o", name=f"o_{s}")
                            nc.vector.tensor_scalar_mul(
                                out=ot[:], in0=zps[:], scalar1=dinvc_sb[:, s : s + 1])
                            nc.sync.dma_start(
                                out=out_dram.ap()[s * P : (s + 1) * P, :], in_=ot[:])

    nc.compile()
    return nc


def kernel(x, edge_index, W0, b0, W1, b1, W2, b2):
    global LAST_EXEC_NS
    import os
    from concourse import bass_utils

    x = np.asarray(x, np.float32)
    ei = np.asarray(edge_index, np.int64)
    row = ei[0].astype(np.int64)
    col = ei[1].astype(np.int64)

    prep = _prep(x, row, col)
    nc = _build_program(prep)

    bf = ml_dtypes.bfloat16
    in_maps = []
    for c in range(NCORES):
        in_maps.append({
            "u0": np.ascontiguousarray(prep["u0"][c]),
            "qidx": np.ascontiguousarray(prep["qidx"][c]),
            "dloc": np.ascontiguousarray(prep["dloc"][c]),
            "W0": np.asarray(W0, np.float32).astype(bf),
            "W1": np.asarray(W1, np.float32).astype(bf),
            "W2": np.asarray(W2, np.float32).astype(bf),
            "b0": np.asarray(b0, np.float32).reshape(1, F).astype(bf),
            "b1": np.asarray(b1, np.float32).reshape(1, F).astype(bf),
            "b2": np.asarray(b2, np.float32).reshape(1, FO).astype(bf),
            "dinvc": np.ascontiguousarray(prep["dinv_cols"][c]),
            "dinv2c": np.ascontiguousarray(prep["dinv2_cols"][c]),
            "rdinv": np.ascontiguousarray(prep["rdinv_rows"][c].reshape(1, SLOTS * P).astype(bf)),
        })

    trace = os.environ.get("GCN_TRACE", "0") == "1"
    if trace:
        _install_ntff_hook()
    res = bass_utils.run_bass_kernel_spmd(
        nc, in_maps, core_ids=list(range(NCORES)), trace=trace)
    LAST_EXEC_NS = res.exec_time_ns

    outs = np.stack([res.results[c]["out"] for c in range(NCORES)])  # [8, 12544, 64]
    core_of, slot_of = prep["core_of"], prep["slot_of"]
    nodes = np.arange(N)
    g_n = nodes >> 7
    out_full = outs[core_of[g_n], slot_of[g_n] * P + (nodes & 127)]
    return out_full.astype(np.float32)


def _install_ntff_hook():
    import types
    try:
        import antenv
        mod = types.ModuleType("antenv.axon_hooks")
        mod._hook = None
        def set_axon_ntff_profile_hook(h):
            mod._hook = h
        def get_axon_ntff_profile_hook():
            return mod._hook
        mod.set_axon_ntff_profile_hook = set_axon_ntff_profile_hook
        mod.get_axon_ntff_profile_hook = get_axon_ntff_profile_hook
        sys.modules["antenv.axon_hooks"] = mod
        antenv.axon_hooks = mod
        from trn_agent_boot.trn_boot import _ntff_profile_via_ctypes
        mod.set_axon_ntff_profile_hook(
            _ntff_profile_via_ctypes("/opt/axon/libaxon_pjrt.so"))
    except Exception as e:
        print(f"NTFF hook install failed: {e}", file=sys.stderr)


# revision 6
# speedup vs baseline: 1.6011x; 1.6011x over previous
"""GCN (3-layer, N=100k nodes, E=1.6M edges) on 8 Trainium2 NeuronCores.

Strategy (self-contained, shapes hardcoded for this problem):
  - Reformulate each layer with associativity:
        h_{l+1} = relu(Dinv (A+I) Dinv h_l W + b)
                = relu(Dinv * [ SpMM(u_l) @ W ] + b),   u_l = Dinv h_l
    and keep u_l = relu(dinv^2 * z + dinv*b) directly (dinv >= 0 commutes
    with relu), so the only per-node scalings are per-partition.
  - Shard the 784 dest-row tiles (128 rows each) across 8 cores, balanced by
    edge count (sorted + round-robin), identical SPMD program on all cores.
  - Each layer: AllGather u shards -> per-core bf16 table [100352, 128] in
    local HBM; gather edge source rows with dma_gather (int16 indices into
    four 25088-row quarters); one-hot selection matrices (iota + is_equal)
    route gathered rows to dest rows via PE matmuls accumulating in PSUM in
    transposed orientation [feat, dest]; then a plain matmul applies W.
"""
import sys
sys.path.insert(0, "/opt/trn_rl_repo")
import numpy as np
import ml_dtypes

N = 100000
F = 128
FO = 64
P = 128
NCORES = 8
SLOTS = 98                 # dest tiles per core
NPAD = NCORES * SLOTS * P  # 100352
NT = NPAD // P             # 784 tiles
QROWS = NPAD // 4          # 25088 rows per gather quarter (< int16 max)
GRP = 6                    # dest tiles processed per PSUM group
import os as _os
KBMAX = int(_os.environ.get("GCN_KBMAX", "32"))   # max 128-edge blocks per dma_gather call
SINGLE_PACKET = _os.environ.get("GCN_SP", "1") == "1"

LAST_EXEC_NS = None


def _prep(x, row, col):
    """Host-side sharding/scheduling. Returns per-core arrays + shared schedule."""
    deg = np.bincount(row, minlength=N).astype(np.float64) + 1.0
    dinv = (1.0 / np.sqrt(deg)).astype(np.float32)

    gt = row >> 7                                   # dest tile of each edge
    cnt = np.bincount(gt, minlength=NT)
    order = np.argsort(-cnt, kind="stable")         # tiles by edge count desc
    core_of = np.empty(NT, np.int64)
    slot_of = np.empty(NT, np.int64)
    core_of[order] = np.tile(np.arange(NCORES), SLOTS)
    slot_of[order] = np.repeat(np.arange(SLOTS), NCORES)

    nodes = np.arange(NPAD)
    g_n = nodes >> 7
    pos = core_of[g_n] * (SLOTS * P) + slot_of[g_n] * P + (nodes & 127)

    dinv_full = np.ones(NPAD, np.float32)
    dinv_full[:N] = dinv

    colpos = pos[col]
    q_e = colpos // QROWS
    qloc_e = (colpos % QROWS).astype(np.int16)
    c_e = core_of[gt]
    s_e = slot_of[gt]
    dloc_e = (row & 127).astype(np.uint8)

    key = (c_e * SLOTS + s_e) * 4 + q_e             # [E]
    sort_idx = np.argsort(key, kind="stable")
    key_s = key[sort_idx]
    cnts = np.bincount(key, minlength=NCORES * SLOTS * 4).reshape(NCORES, SLOTS, 4)
    B = -(-cnts // P)                               # blocks needed [core, slot, q]
    Bq = B.max(axis=0)                              # shared schedule [slot, q]

    # global block emission order: groups of GRP slots, quarter-major inside
    groups = [list(range(g, min(g + GRP, SLOTS))) for g in range(0, SLOTS, GRP)]
    blkoff = np.zeros((SLOTS, 4), np.int64)
    emit = []                                       # [(slot, q)] in block order
    off = 0
    sched = []   # per group: dict(slots=[...], qcalls={q: [(b0, nb, [(slot,last)..])]})
    slot_nblocks = Bq.sum(axis=1)
    for grp in groups:
        qcalls = {}
        for q in range(4):
            blocks = []                             # (slot, islast) per block
            for s in grp:
                nb_s = Bq[s, q]
                if nb_s == 0:
                    continue
                blkoff[s, q] = off + len(blocks)
                done_before = Bq[s, :q].sum()
                for j in range(nb_s):
                    islast = (done_before + j) == slot_nblocks[s] - 1
                    blocks.append((s, islast))
            calls = []
            b0 = 0
            while b0 < len(blocks):
                nb = min(KBMAX, len(blocks) - b0)
                calls.append((off + b0, nb, blocks[b0 : b0 + nb]))
                b0 += nb
            qcalls[q] = calls
            off += len(blocks)
        sched.append({"slots": grp, "qcalls": qcalls})
    TB = int(off)

    # per-core lane arrays
    qidx_lanes = np.zeros((NCORES, TB * P), np.int16)
    dloc_lanes = np.full((NCORES, TB * P), 255, np.int16)
    starts = np.zeros(NCORES * SLOTS * 4 + 1, np.int64)
    np.cumsum(np.bincount(key, minlength=NCORES * SLOTS * 4), out=starts[1:])
    rank_in_seg = np.arange(len(key_s)) - starts[key_s]
    c_s = key_s // (SLOTS * 4)
    s_s = (key_s // 4) % SLOTS
    q_s = key_s % 4
    blk = blkoff[s_s, q_s] + rank_in_seg // P
    lane = rank_in_seg % P
    dst = blk * P + lane
    e_ids = sort_idx
    qidx_lanes[c_s, dst] = qloc_e[e_ids]
    dloc_lanes[c_s, dst] = dloc_e[e_ids]

    # pack indices per call: [128, 8*TB] with 16-partition wrap per call
    def pack_core(qi):
        colsets = []
        for grp_s in sched:
            for q in range(4):
                for (b0, nb, _blocks) in grp_s["qcalls"][q]:
                    seg = qi[b0 * P : (b0 + nb) * P]
                    arr = seg.reshape(-1, 16).T          # [16, 8*nb]
                    colsets.append(np.tile(arr, (8, 1)))
        return np.concatenate(colsets, axis=1).astype(np.int16)

    qidx_packed = np.stack([pack_core(qidx_lanes[c]) for c in range(NCORES)])
    dloc_in = (
        dloc_lanes.reshape(NCORES, TB, P)
        .transpose(0, 2, 1)
        .astype(np.float32)
        .astype(ml_dtypes.bfloat16)
    )                                                # [NCORES, 128, TB]

    # per-core u0 shard, dinv columns
    x_pad = np.zeros((NPAD, F), np.float32)
    x_pad[:N] = x
    u0_full = (dinv_full[:, None] * x_pad).astype(ml_dtypes.bfloat16)
    u0_shards = np.empty((NCORES, SLOTS * P, F), ml_dtypes.bfloat16)
    u0_shards.reshape(NCORES, SLOTS * P, F)[core_of[g_n], slot_of[g_n] * P + (nodes & 127)] = u0_full

    dv = np.empty((NCORES, SLOTS * P), np.float32)
    dv[core_of[g_n], slot_of[g_n] * P + (nodes & 127)] = dinv_full
    dinv_cols = dv.reshape(NCORES, SLOTS, P).transpose(0, 2, 1).copy()   # [c,128,SLOTS]
    dinv2_cols = (dinv_cols ** 2).copy()
    rdinv_rows = (1.0 / dv.reshape(NCORES, SLOTS, P)).astype(np.float32) # [c,SLOTS,128]

    return dict(
        sched=sched, TB=TB, Bq=Bq,
        qidx=qidx_packed, dloc=dloc_in, u0=u0_shards,
        dinv_cols=dinv_cols, dinv2_cols=dinv2_cols, rdinv_rows=rdinv_rows,
        core_of=core_of, slot_of=slot_of,
    )


def _build_program(prep, trace=False):
    import os
    from concourse import bass, bacc, mybir, tile
    from concourse.masks import make_identity
    NLAYERS = int(os.environ.get("GCN_LAYERS", "3"))
    NOGATHER = os.environ.get("GCN_NOGATHER", "0") == "1"
    NOCC = os.environ.get("GCN_NOCC", "0") == "1"

    bf16 = mybir.dt.bfloat16
    f32 = mybir.dt.float32

    sched = prep["sched"]
    TB = prep["TB"]

    NSWQ = int(os.environ.get("GCN_NSWQ", "1"))
    nc = bacc.Bacc("TRN2", target_bir_lowering=False, debug=False, num_devices=NCORES,
                   num_swdge_queues=NSWQ)

    u0_in = nc.dram_tensor("u0", [SLOTS * P, F], bf16, kind="ExternalInput")
    qidx_in = nc.dram_tensor("qidx", [P, 8 * TB], mybir.dt.int16, kind="ExternalInput")
    dloc_in = nc.dram_tensor("dloc", [P, TB], bf16, kind="ExternalInput")
    w_ins = [
        nc.dram_tensor("W0", [F, F], bf16, kind="ExternalInput"),
        nc.dram_tensor("W1", [F, F], bf16, kind="ExternalInput"),
        nc.dram_tensor("W2", [F, FO], bf16, kind="ExternalInput"),
    ]
    b_ins = [
        nc.dram_tensor("b0", [1, F], bf16, kind="ExternalInput"),
        nc.dram_tensor("b1", [1, F], bf16, kind="ExternalInput"),
        nc.dram_tensor("b2", [1, FO], bf16, kind="ExternalInput"),
    ]
    dinvc_in = nc.dram_tensor("dinvc", [P, SLOTS], f32, kind="ExternalInput")
    dinv2c_in = nc.dram_tensor("dinv2c", [P, SLOTS], f32, kind="ExternalInput")
    rdinv_in = nc.dram_tensor("rdinv", [1, SLOTS * P], bf16, kind="ExternalInput")
    out_dram = nc.dram_tensor("out", [SLOTS * P, FO], f32, kind="ExternalOutput")

    with tile.TileContext(nc) as tc:
        with tc.tile_pool(name="dram", bufs=1, space="DRAM") as dram, \
             tc.tile_pool(name="consts", bufs=1) as consts, \
             tc.tile_pool(name="uarena", bufs=1) as uarena, \
             tc.tile_pool(name="gpool", bufs=int(_os.environ.get("GCN_GBUFS","4"))) as gpool, \
             tc.tile_pool(name="spool", bufs=int(_os.environ.get("GCN_SBUFS","4"))) as spool, \
             tc.tile_pool(name="uown0", bufs=3) as uown0, \
             tc.tile_pool(name="ytpool", bufs=3) as ytpool, \
             tc.tile_pool(name="opool", bufs=3) as opool, \
             tc.tile_pool(name="spmmps", bufs=GRP, space="PSUM") as spmmps, \
             tc.tile_pool(name="zpsp", bufs=2, space="PSUM") as zpsp:

            # ---- constants ----
            ident = consts.tile([P, P], bf16)
            make_identity(nc, ident[:])
            iota_i = consts.tile([P, P], mybir.dt.int32)
            nc.gpsimd.iota(iota_i[:], pattern=[[1, P]], base=0, channel_multiplier=0)
            iota_b = consts.tile([P, P], bf16)
            nc.vector.tensor_copy(out=iota_b[:], in_=iota_i[:])
            qidx_sb = consts.tile([P, 8 * TB], mybir.dt.int16)
            nc.sync.dma_start(out=qidx_sb[:], in_=qidx_in.ap()[:])
            dloc_sb = consts.tile([P, TB], bf16)
            nc.sync.dma_start(out=dloc_sb[:], in_=dloc_in.ap()[:])
            w_sb = []
            for li, w in enumerate(w_ins):
                wt = consts.tile(list(w.shape), bf16, name=f"w{li}sb")
                nc.sync.dma_start(out=wt[:], in_=w.ap()[:])
                w_sb.append(wt)
            b_sb = []
            for li, b in enumerate(b_ins):
                bt = consts.tile(list(b.shape), bf16, name=f"b{li}sb")
                nc.sync.dma_start(out=bt[:], in_=b.ap()[:])
                b_sb.append(bt)
            dinvc_sb = consts.tile([P, SLOTS], f32)
            nc.sync.dma_start(out=dinvc_sb[:], in_=dinvc_in.ap()[:])
            dinv2c_sb = consts.tile([P, SLOTS], f32)
            nc.sync.dma_start(out=dinv2c_sb[:], in_=dinv2c_in.ap()[:])
            rdinv_sb = consts.tile([1, SLOTS * P], bf16)
            nc.sync.dma_start(out=rdinv_sb[:], in_=rdinv_in.ap()[:])

            # ---- u arenas (layer outputs kept resident in SBUF) ----
            uA = [uarena.tile([P, F], bf16, tag=f"uA{s}", name=f"uA{s}") for s in range(SLOTS)]
            uB = [uarena.tile([P, F], bf16, tag=f"uB{s}", name=f"uB{s}") for s in range(SLOTS)]

            # ---- AG staging + tables ----
            stages = [dram.tile([SLOTS * P, F], bf16, name=f"stage{l}") for l in range(3)]
            tables = [dram.tile([NPAD, F], bf16, addr_space="Shared", name=f"table{l}")
                      for l in range(3)]

            # layer 0 staging comes straight from the input shard
            nc.sync.dma_start(out=stages[0][:], in_=u0_in.ap()[:])

            rg = [list(range(NCORES))]
            for layer in range(NLAYERS):
                last = layer == NLAYERS - 1
                fo = FO if last else F
                u_in_tiles = None if layer == 0 else (uA if layer == 1 else uB)
                u_out_tiles = uA if layer == 0 else (uB if layer == 1 else None)

                if not NOCC:
                    nc.gpsimd.collective_compute(
                        "AllGather", mybir.AluOpType.bypass,
                        replica_groups=rg,
                        ins=[stages[layer].opt()], outs=[tables[layer].opt()],
                    )
                else:
                    nc.sync.dma_start(
                        out=tables[layer].opt()[:SLOTS * P, :],
                        in_=stages[layer].opt()[:])

                for grp_s in sched:
                    psums = {}
                    uowns = {}
                    for s in grp_s["slots"]:
                        if layer == 0:
                            ut = uown0.tile([P, F], bf16, tag="u0own", name=f"u0own_{layer}_{s}")
                            nc.sync.dma_start(
                                out=ut[:], in_=u0_in.ap()[s * P : (s + 1) * P, :])
                            uowns[s] = ut
                        else:
                            uowns[s] = u_in_tiles[s]
                        ps = spmmps.tile([P, P], f32, tag="spmm", name=f"ps_{layer}_{s}",
                                         space="PSUM")
                        psums[s] = ps
                        # self-loop: psum[fi, dest] += u_own.T
                        nc.tensor.matmul(
                            out=ps[:], lhsT=uowns[s][:], rhs=ident[:],
                            start=True,
                            stop=(NOGATHER or int(prep["Bq"][s].sum()) == 0),
                        )
                    _callctr = [0]
                    for q in ([] if NOGATHER else range(4)):
                        for (b0, nb, blocks) in grp_s["qcalls"][q]:
                            gt_ = gpool.tile([P, nb, F], bf16, tag="g", name=f"g_{layer}_{b0}")
                            nc.gpsimd.dma_gather(
                                out_ap=gt_[:],
                                in_ap=tables[layer].opt()[q * QROWS : (q + 1) * QROWS, :],
                                idxs_ap=qidx_sb[:, 8 * b0 : 8 * (b0 + nb)],
                                num_idxs=nb * P,
                                num_idxs_reg=nb * P,
                                elem_size=F,
                                single_packet=SINGLE_PACKET,
                                queue_num=_callctr[0] % NSWQ,
                            )
                            _callctr[0] += 1
                            st_ = spool.tile([P, nb, P], bf16, tag="s", name=f"s_{layer}_{b0}")
                            nc.vector.tensor_tensor(
                                out=st_[:],
                                in0=dloc_sb[:, b0 : b0 + nb]
                                    .rearrange("p (a b) -> p a b", b=1)
                                    .to_broadcast([P, nb, P]),
                                in1=iota_b[:]
                                    .rearrange("p (a b) -> p a b", a=1)
                                    .to_broadcast([P, nb, P]),
                                op=mybir.AluOpType.is_equal,
                            )
                            for j, (s, islast) in enumerate(blocks):
                                nc.tensor.matmul(
                                    out=psums[s][:],
                                    lhsT=gt_[:, j, :], rhs=st_[:, j, :],
                                    start=False, stop=islast,
                                )
                    for s in grp_s["slots"]:
                        yt = ytpool.tile([P, P], bf16, tag="yt", name=f"yt_{layer}_{s}")
                        nc.vector.tensor_copy(out=yt[:], in_=psums[s][:])
                        zps = zpsp.tile([P, fo], f32, tag="z", name=f"z_{layer}_{s}",
                                        space="PSUM")
                        nc.tensor.matmul(out=zps[:],
                                         lhsT=yt[:], rhs=w_sb[2 if last else layer][:],
                                         start=True, stop=False)
                        nc.tensor.matmul(out=zps[:], lhsT=rdinv_sb[0:1, s * P : (s + 1) * P],
                                         rhs=b_sb[2 if last else layer][:], start=False, stop=True)
                        if not last:
                            un = u_out_tiles[s]
                            nc.scalar.activation(
                                out=un[:], in_=zps[:],
                                func=mybir.ActivationFunctionType.Relu,
                                scale=dinv2c_sb[:, s : s + 1],
                            )
                            nc.sync.dma_start(
                                out=stages[layer + 1][s * P : (s + 1) * P, :],
                                in_=un[:])
                        else:
                            ot = opool.tile([P, FO], f32, tag="o", name=f"o_{s}")
                            nc.vector.tensor_scalar_mul(
                                out=ot[:], in0=zps[:], scalar1=dinvc_sb[:, s : s + 1])
                            nc.sync.dma_start(
                                out=out_dram.ap()[s * P : (s + 1) * P, :], in_=ot[:])

    nc.compile()
    return nc


def kernel(x, edge_index, W0, b0, W1, b1, W2, b2):
    global LAST_EXEC_NS
    import os
    from concourse import bass_utils

    x = np.asarray(x, np.float32)
    ei = np.asarray(edge_index, np.int64)
    row = ei[0].astype(np.int64)
    col = ei[1].astype(np.int64)

    prep = _prep(x, row, col)
    nc = _build_program(prep)

    bf = ml_dtypes.bfloat16
    in_maps = []
    for c in range(NCORES):
        in_maps.append({
            "u0": np.ascontiguousarray(prep["u0"][c]),
            "qidx": np.ascontiguousarray(prep["qidx"][c]),
            "dloc": np.ascontiguousarray(prep["dloc"][c]),
            "W0": np.asarray(W0, np.float32).astype(bf),
            "W1": np.asarray(W1, np.float32).astype(bf),
            "W2": np.asarray(W2, np.float32).astype(bf),
            "b0": np.asarray(b0, np.float32).reshape(1, F).astype(bf),
            "b1": np.asarray(b1, np.float32).reshape(1, F).astype(bf),
            "b2": np.asarray(b2, np.float32).reshape(1, FO).astype(bf),
            "dinvc": np.ascontiguousarray(prep["dinv_cols"][c]),
            "dinv2c": np.ascontiguousarray(prep["dinv2_cols"][c]),
            "rdinv": np.ascontiguousarray(prep["rdinv_rows"][c].reshape(1, SLOTS * P).astype(bf)),
        })

    trace = os.environ.get("GCN_TRACE", "0") == "1"
    if trace:
        _install_ntff_hook()
    res = bass_utils.run_bass_kernel_spmd(
        nc, in_maps, core_ids=list(range(NCORES)), trace=trace)
    LAST_EXEC_NS = res.exec_time_ns

    outs = np.stack([res.results[c]["out"] for c in range(NCORES)])  # [8, 12544, 64]
    core_of, slot_of = prep["core_of"], prep["slot_of"]
    nodes = np.arange(N)
    g_n = nodes >> 7
    out_full = outs[core_of[g_n], slot_of[g_n] * P + (nodes & 127)]
    return out_full.astype(np.float32)


def _install_ntff_hook():
    import types
    try:
        import antenv
        mod = types.ModuleType("antenv.axon_hooks")
        mod._hook = None
        def set_axon_ntff_profile_hook(h):
            mod._hook = h
        def get_axon_ntff_profile_hook():
            return mod._hook
        mod.set_axon_ntff_profile_hook = set_axon_ntff_profile_hook
        mod.get_axon_ntff_profile_hook = get_axon_ntff_profile_hook
        sys.modules["antenv.axon_hooks"] = mod
        antenv.axon_hooks = mod
        from trn_agent_boot.trn_boot import _ntff_profile_via_ctypes
        mod.set_axon_ntff_profile_hook(
            _ntff_profile_via_ctypes("/opt/axon/libaxon_pjrt.so"))
    except Exception as e:
        print(f"NTFF hook install failed: {e}", file=sys.stderr)
